# revision 1
# baseline (speedup 1.0000x reference)
"""Trainium2 Bass kernel for nn_DenseNetCmaxGatedB2 (gated pooling block).

Computation (per batch, per channel, depthwise):
  out = maxpool3x3s2(x) * (dwconv_s2(x, maxgate) + mb)
  g0  = sigmoid(dwconv_s2(x, pgates[...,0]) + gbs[:,0])
  n0  = g0*p0 + (1-g0)*p1           p_k = dwconv_s2(x, pconvs[...,k]) + pbs[:,k]
  g1  = sigmoid(dwconv_s2(x, pgates[...,2]) + gbs[:,1])
  n1  = g1*p2 + (1-g1)*p3
  g   = sigmoid(dwconv_s1(n0, pgates[...,2]) + gbs[:,2])
  out = out + n0*g + n1*(1-g)

Sharding: pure data parallel over batch (16 -> 2 per core, 8 cores).

Layout: channels on SBUF partitions (128 per plane; 2 batches x 2
channel-blocks = 4 planes per core).  For bf16 compute, x is
deinterleaved once per plane into even/odd row x col parity planes so
every tap reads with unit stride (required for the DVE 2x/4x perf
modes).  Each stride-2 conv tap is a tensor_scalar product (4x bf16
mode, per-channel weight as the per-partition fp32 scalar) plus a
tensor_tensor accumulate (2x mode) — scalar_tensor_tensor would fuse
both but only has a 1x DVE uop, which measures slower.
TensorScalarPtr / TensorTensor are not legal Pool-engine opcodes on
TRN2 (walrus ISA check), so all elementwise compute runs on VectorE;
ScalarE does the deinterleave, the first tap of each conv (fused
scale+bias via activation Identity) and the sigmoids.

Measured (8-core SPMD, per-core HW time via For_i repeat-loop wall
deltas): ~1.31-1.41 ms with all products on DVE; rel err (absmax-
scaled) ~1.1e-2 vs fp32 ref.  Final version alternates tap products
between ScalarE and DVE (cost model: 937 us vs 1093 us) — numerically
identical ops, only engine placement differs.
"""

import contextlib
import sys

sys.path.insert(0, "/opt/trn_rl_repo")

import numpy as np

import concourse.bass as bass  # noqa: E402,F401
import concourse.mybir as mybir  # noqa: E402
from concourse import bacc  # noqa: E402
from concourse.tile import TileContext  # noqa: E402
from concourse.bass_utils import run_bass_kernel_spmd  # noqa: E402

N_CORES = 8
B, C, H = 16, 256, 128
HO = H // 2
BS = B // N_CORES  # batches per core
F32 = mybir.dt.float32
BF16 = mybir.dt.bfloat16
AF = mybir.ActivationFunctionType
OP = mybir.AluOpType

# (di, dj) tap order; (1,1) handled by ScalarE with fused scale+bias.
TAPS8 = [(0, 0), (0, 1), (0, 2), (1, 0), (1, 2), (2, 0), (2, 1), (2, 2)]


def _build(dt, reps=1):
    """Build the SPMD program for one core (2 batches, full channels).

    reps>1 wraps the per-plane pipeline in a hardware loop recomputing
    the same outputs; used only for wall-clock timing."""
    nc = bacc.Bacc("TRN2", target_bir_lowering=False, debug=False, num_devices=N_CORES)

    x_d = nc.dram_tensor("x", [BS, C, H * H], F32, kind="ExternalInput")
    mg_d = nc.dram_tensor("maxgate", [C, 9], F32, kind="ExternalInput")
    mb_d = nc.dram_tensor("mb", [C, 1], F32, kind="ExternalInput")
    pc_d = nc.dram_tensor("pconvs", [C, 36], F32, kind="ExternalInput")
    pb_d = nc.dram_tensor("pbs", [C, 4], F32, kind="ExternalInput")
    pg_d = nc.dram_tensor("pgates", [C, 27], F32, kind="ExternalInput")
    gb_d = nc.dram_tensor("gbs", [C, 3], F32, kind="ExternalInput")
    out_d = nc.dram_tensor("out", [BS, C, HO * HO], F32, kind="ExternalOutput")

    bf = dt == BF16
    V = nc.vector

    with TileContext(nc) as tc:
        with contextlib.ExitStack() as ctx:
            wp = ctx.enter_context(tc.tile_pool(name="w", bufs=1))
            xp = ctx.enter_context(tc.tile_pool(name="xp", bufs=1))
            pp = ctx.enter_context(tc.tile_pool(name="pp", bufs=2))
            ppz = ctx.enter_context(tc.tile_pool(name="ppz", bufs=1))
            ap = ctx.enter_context(tc.tile_pool(name="ap", bufs=1))
            op_ = ctx.enter_context(tc.tile_pool(name="op", bufs=2))

            # ---- weights / biases (fp32 per-partition scalars), per cblock
            W = []
            for cb in range(2):
                sl = slice(cb * 128, (cb + 1) * 128)
                wmg = wp.tile([128, 9], F32, tag=f"wmg{cb}")
                wpc = wp.tile([128, 36], F32, tag=f"wpc{cb}")
                wpg = wp.tile([128, 27], F32, tag=f"wpg{cb}")
                bmb = wp.tile([128, 1], F32, tag=f"bmb{cb}")
                bpb = wp.tile([128, 4], F32, tag=f"bpb{cb}")
                bgb = wp.tile([128, 3], F32, tag=f"bgb{cb}")
                nc.sync.dma_start(wmg[:], mg_d[sl, :])
                nc.sync.dma_start(wpc[:], pc_d[sl, :])
                nc.sync.dma_start(wpg[:], pg_d[sl, :])
                nc.sync.dma_start(bmb[:], mb_d[sl, :])
                nc.sync.dma_start(bpb[:], pb_d[sl, :])
                nc.sync.dma_start(bgb[:], gb_d[sl, :])

                def s(t, i):
                    return t[:, i : i + 1]

                def mk(wt, stride_, k):
                    return lambda di, dj, wt=wt, stride_=stride_, k=k: s(
                        wt, (di * 3 + dj) * stride_ + k
                    )

                W.append(
                    dict(
                        cm=(mk(wmg, 1, 0), s(bmb, 0)),
                        g0=(mk(wpg, 3, 0), s(bgb, 0)),
                        p0=(mk(wpc, 4, 0), s(bpb, 0)),
                        p1=(mk(wpc, 4, 1), s(bpb, 1)),
                        g1=(mk(wpg, 3, 2), s(bgb, 1)),
                        p2=(mk(wpc, 4, 2), s(bpb, 2)),
                        p3=(mk(wpc, 4, 3), s(bpb, 3)),
                        nd=(mk(wpg, 3, 2), s(bgb, 2)),
                    )
                )

            tmp_pool = ctx.enter_context(tc.tile_pool(name="tmp", bufs=2))

            def conv_s2(acc3, planes, wfn, bias):
                """Stride-2 3x3 depthwise conv into acc3 [128,64,64].

                scalar_tensor_tensor only has a 1x DVE uop, so instead each
                tap is a tensor_scalar product (4x mode in bf16) plus a
                tensor_tensor accumulate (2x mode) — ~35% fewer DVE cycles
                than the 1x fused MAC."""
                nc.scalar.activation(
                    acc3, planes["ee"][:], AF.Identity, bias=bias, scale=wfn(1, 1)
                )
                for di, dj in TAPS8:
                    rsel = "e" if di == 1 else "o"
                    csel = {0: "z", 1: "e", 2: "o"}[dj]
                    p = planes[rsel + csel]
                    i0 = 1 if di == 0 else 0
                    pin = p[:, 0 : 64 - i0, 0:64]
                    po = acc3[:, i0:64, :]
                    t = tmp_pool.tile([128, 64, 64], dt, tag="t", bufs=3, name="t")
                    tv = t[:, 0 : 64 - i0, :]
                    # alternate products between ScalarE (otherwise mostly
                    # idle) and DVE tensor_scalar (4x bf16); adds stay on DVE
                    if (di + dj) % 2 == 0:
                        nc.scalar.mul(tv, pin, wfn(di, dj))
                    else:
                        V.tensor_scalar(tv, pin, wfn(di, dj), None, OP.mult)
                    V.tensor_tensor(po, po, tv, OP.add)

            def conv_s2_strided(acc3, xv, wfn, bias):
                """fp32 path: taps read x [128,128,128] directly (strided)."""
                nc.scalar.activation(
                    acc3, xv[:, 0:128:2, 0:128:2], AF.Identity, bias=bias,
                    scale=wfn(1, 1),
                )
                for di, dj in TAPS8:
                    i0 = 1 if di == 0 else 0
                    j0 = 1 if dj == 0 else 0
                    r0 = di - 1 + 2 * i0
                    c0 = dj - 1 + 2 * j0
                    pin = xv[:, r0:128:2, c0:128:2][:, 0 : 64 - i0, 0 : 64 - j0]
                    po = acc3[:, i0:64, j0:64]
                    V.scalar_tensor_tensor(po, pin, wfn(di, dj), po, OP.mult, OP.add)

            def plane(b, cb):
                sl = slice(cb * 128, (cb + 1) * 128)
                w = W[cb]

                X = xp.tile([128, H * H], dt, tag="X", name="X")
                if bf:
                    nc.gpsimd.dma_start(X[:], x_d[b, sl, :])  # casts f32->bf16
                else:
                    nc.sync.dma_start(X[:], x_d[b, sl, :])
                xv = X[:].rearrange("p (r c) -> p r c", r=H)

                planes = None
                if bf:
                    pee = pp.tile([128, 64, 64], dt, tag="pee", name="pee")
                    peo = pp.tile([128, 64, 64], dt, tag="peo", name="peo")
                    poe = pp.tile([128, 64, 64], dt, tag="poe", name="poe")
                    poo = pp.tile([128, 64, 64], dt, tag="poo", name="poo")
                    pez = ppz.tile([128, 64, 65], dt, tag="pez", name="pez")
                    poz = ppz.tile([128, 64, 65], dt, tag="poz", name="poz")
                    nc.scalar.copy(pee[:], xv[:, 0:128:2, 0:128:2])
                    nc.scalar.copy(peo[:], xv[:, 0:128:2, 1:128:2])
                    nc.scalar.copy(poe[:], xv[:, 1:128:2, 0:128:2])
                    nc.scalar.copy(poo[:], xv[:, 1:128:2, 1:128:2])
                    nc.gpsimd.memset(pez[:, :, 0:1], 0)
                    nc.gpsimd.memset(poz[:, :, 0:1], 0)
                    nc.scalar.copy(pez[:, :, 1:65], xv[:, 0:128:2, 1:128:2])
                    nc.scalar.copy(poz[:, :, 1:65], xv[:, 1:128:2, 1:128:2])
                    planes = dict(ee=pee, eo=peo, oe=poe, oo=poo, ez=pez, oz=poz)

                def conv(acc3, key):
                    wfn, bias = w[key]
                    if bf:
                        conv_s2(acc3, planes, wfn, bias)
                    else:
                        conv_s2_strided(acc3, xv, wfn, bias)

                cm = ap.tile([128, 64, 64], dt, tag="A", name="cm")
                conv(cm[:], "cm")

                # maxpool via tensor_tensor max chain
                mp = ap.tile([128, 64, 64], dt, tag="B", name="mp")
                m3 = mp[:]
                rest = [(0, 0), (0, 1), (0, 2), (1, 0), (2, 0), (2, 1), (2, 2)]
                if bf:
                    V.tensor_tensor(m3, planes["ee"][:], planes["eo"][:], OP.max)
                    for di, dj in rest:
                        rsel = "e" if di == 1 else "o"
                        csel = {0: "o", 1: "e", 2: "o"}[dj]
                        p = planes[rsel + csel]
                        i0 = 1 if di == 0 else 0
                        j0 = 1 if dj == 0 else 0
                        pin = p[:, 0 : 64 - i0, 0 : 64 - j0]
                        po = m3[:, i0:64, j0:64]
                        V.tensor_tensor(po, po, pin, OP.max)
                else:
                    V.tensor_tensor(
                        m3, xv[:, 0:128:2, 0:128:2], xv[:, 0:128:2, 1:128:2], OP.max
                    )
                    for di, dj in rest:
                        i0 = 1 if di == 0 else 0
                        j0 = 1 if dj == 0 else 0
                        r0 = di - 1 + 2 * i0
                        c0 = dj - 1 + 2 * j0
                        pin = xv[:, r0:128:2, c0:128:2][:, 0 : 64 - i0, 0 : 64 - j0]
                        po = m3[:, i0:64, j0:64]
                        V.tensor_tensor(po, po, pin, OP.max)

                # mpcm = maxpool * cm   (keep in B)
                V.tensor_tensor(m3, m3, cm[:], OP.mult)

                g0 = ap.tile([128, 64, 64], dt, tag="A2", name="g0")
                conv(g0[:], "g0")
                nc.scalar.activation(g0[:], g0[:], AF.Sigmoid)

                p0 = ap.tile([128, 64, 64], dt, tag="C", name="p0")
                conv(p0[:], "p0")
                p1 = ap.tile([128, 64, 64], dt, tag="D", name="p1")
                conv(p1[:], "p1")

                # n0 = p1 + g0*(p0-p1), stored zero-padded [64,66]
                n0z = ap.tile([128, 64, 66], dt, tag="E", name="n0z")
                V.tensor_tensor(p0[:], p0[:], p1[:], OP.subtract)
                V.tensor_tensor(p0[:], p0[:], g0[:], OP.mult)
                nc.gpsimd.memset(n0z[:, :, 0:1], 0)
                nc.gpsimd.memset(n0z[:, :, 65:66], 0)
                n0 = n0z[:, :, 1:65]
                V.tensor_tensor(n0, p0[:], p1[:], OP.add)

                g1 = ap.tile([128, 64, 64], dt, tag="A2", name="g1")
                conv(g1[:], "g1")
                nc.scalar.activation(g1[:], g1[:], AF.Sigmoid)
                p2 = ap.tile([128, 64, 64], dt, tag="C", name="p2")
                conv(p2[:], "p2")
                p3 = ap.tile([128, 64, 64], dt, tag="D", name="p3")
                conv(p3[:], "p3")

                V.tensor_tensor(p2[:], p2[:], p3[:], OP.subtract)
                V.tensor_tensor(p2[:], p2[:], g1[:], OP.mult)
                V.tensor_tensor(p2[:], p2[:], p3[:], OP.add)
                n1 = p2  # tag C

                # node-stage gate: stride-1 conv over padded n0
                gc = ap.tile([128, 64, 64], dt, tag="A2", name="gc")
                wfn, bias = w["nd"]
                nc.scalar.activation(
                    gc[:], n0z[:, 0:64, 1:65], AF.Identity, bias=bias, scale=wfn(1, 1)
                )
                for di, dj in TAPS8:
                    i0 = 1 if di == 0 else 0
                    i1 = 1 if di == 2 else 0
                    r0n = di - 1 + i0
                    pin = n0z[:, r0n : r0n + 64 - i0 - i1, dj : dj + 64]
                    po = gc[:, i0 : 64 - i1, :]
                    t = tmp_pool.tile([128, 64, 64], dt, tag="t", bufs=3, name="t")
                    tv = t[:, 0 : 64 - i0 - i1, :]
                    if (di + dj) % 2 == 0:
                        nc.scalar.mul(tv, pin, wfn(di, dj))
                    else:
                        V.tensor_scalar(tv, pin, wfn(di, dj), None, OP.mult)
                    V.tensor_tensor(po, po, tv, OP.add)
                nc.scalar.activation(gc[:], gc[:], AF.Sigmoid)

                # out = mpcm + n1 + g*(n0-n1)
                o = op_.tile([128, 64, 64], dt, tag="O", name="o")
                V.tensor_tensor(o[:], n0, n1[:], OP.subtract)
                V.tensor_tensor(o[:], o[:], gc[:], OP.mult)
                V.tensor_tensor(o[:], o[:], n1[:], OP.add)
                V.tensor_tensor(o[:], o[:], m3, OP.add)

                oflat = o[:].rearrange("p a b -> p (a b)")
                if bf:
                    nc.gpsimd.dma_start(out_d[b, sl, :], oflat)  # cast back
                else:
                    nc.sync.dma_start(out_d[b, sl, :], oflat)

            rep_ctx = tc.For_i(0, reps, 1) if reps > 1 else contextlib.nullcontext()
            with rep_ctx:
                for b in range(BS):
                    for cb in range(2):
                        plane(b, cb)

    nc.compile()
    return nc


_NC_CACHE = {}


def _get_nc(dt, reps=1):
    key = (str(dt), reps)
    if key not in _NC_CACHE:
        _NC_CACHE[key] = _build(dt, reps)
    return _NC_CACHE[key]


def _in_maps(x, maxgate, mb, pconvs, pbs, pgates, gbs):
    x = np.ascontiguousarray(np.asarray(x, np.float32))
    maps = []
    for i in range(N_CORES):
        maps.append(
            dict(
                x=x[i * BS : (i + 1) * BS].reshape(BS, C, H * H),
                maxgate=np.asarray(maxgate, np.float32).reshape(C, 9),
                mb=np.asarray(mb, np.float32).reshape(C, 1),
                pconvs=np.asarray(pconvs, np.float32).reshape(C, 36),
                pbs=np.asarray(pbs, np.float32).reshape(C, 4),
                pgates=np.asarray(pgates, np.float32).reshape(C, 27),
                gbs=np.asarray(gbs, np.float32).reshape(C, 3),
            )
        )
    return maps


def kernel(x, maxgate, mb, pconvs, pbs, pgates, gbs):
    nc = _get_nc(BF16)
    maps = _in_maps(x, maxgate, mb, pconvs, pbs, pgates, gbs)
    res = run_bass_kernel_spmd(nc, maps, list(range(N_CORES)))
    return np.concatenate(
        [r["out"].reshape(BS, C, HO, HO) for r in res.results], axis=0
    )



# revision 2
# speedup vs baseline: 1.4318x; 1.4318x over previous
"""Trainium2 Bass kernel v2 for nn_DenseNetCmaxGatedB2 (gated pooling block).

Computation (per batch, per channel, depthwise):
  out = maxpool3x3s2(x) * (dwconv_s2(x, maxgate) + mb)
  g0  = sigmoid(dwconv_s2(x, pgates[...,0]) + gbs[:,0])
  n0  = g0*p0 + (1-g0)*p1           p_k = dwconv_s2(x, pconvs[...,k]) + pbs[:,k]
  g1  = sigmoid(dwconv_s2(x, pgates[...,2]) + gbs[:,1])
  n1  = g1*p2 + (1-g1)*p3
  g   = sigmoid(dwconv_s1(n0, pgates[...,2]) + gbs[:,2])
  out = out + n0*g + n1*(1-g)

v2 strategy: all conv tap-MACs run on the TensorEngine as diagonal-weight
matmuls accumulating in fp32 PSUM (9 taps per conv; per-channel weight on
the diagonal of a 128x128 bf16 stationary matrix).  ScalarE evacuates
PSUM with fused bias (+sigmoid for gates) to bf16 SBUF.  DVE does the
maxpool (tensor_tensor max chain on parity planes) and gating combines.
Weight-linearity trick: n0 = p1 + g0*conv(x, w0-w1), so only 7 stride-2
convs + 1 stride-1 conv are needed.

Sharding: pure data parallel over batch (16 -> 2 per core, 8 cores).
"""

import contextlib
import sys

sys.path.insert(0, "/opt/trn_rl_repo")

import numpy as np

import concourse.bass as bass  # noqa: E402,F401
import concourse.mybir as mybir  # noqa: E402
from concourse import bacc  # noqa: E402
from concourse.tile import TileContext  # noqa: E402
from concourse.bass_utils import run_bass_kernel_spmd  # noqa: E402

N_CORES = 8
B, C, H = 16, 256, 128
HO = H // 2
BS = B // N_CORES  # batches per core
F32 = mybir.dt.float32
BF16 = mybir.dt.bfloat16
AF = mybir.ActivationFunctionType
OP = mybir.AluOpType

# conv definitions: name -> (weight source, bias source, sigmoid?)
# weight tap vectors are [128, 9] tiles built at setup per channel block.
CONVS = ["g0", "d01", "p1", "g1", "d23", "p3", "cm"]


def _build(dt=BF16, reps=1):
    """Build the SPMD program for one core (2 batches, full channels)."""
    nc = bacc.Bacc("TRN2", target_bir_lowering=False, debug=False, num_devices=N_CORES)

    x_d = nc.dram_tensor("x", [BS, C, H, H], F32, kind="ExternalInput")
    mg_d = nc.dram_tensor("maxgate", [C, 9], F32, kind="ExternalInput")
    mb_d = nc.dram_tensor("mb", [C, 1], F32, kind="ExternalInput")
    pc_d = nc.dram_tensor("pconvs", [C, 36], F32, kind="ExternalInput")
    pb_d = nc.dram_tensor("pbs", [C, 4], F32, kind="ExternalInput")
    pg_d = nc.dram_tensor("pgates", [C, 27], F32, kind="ExternalInput")
    gb_d = nc.dram_tensor("gbs", [C, 3], F32, kind="ExternalInput")
    out_d = nc.dram_tensor("out", [BS, C, HO * HO], F32, kind="ExternalOutput")

    V = nc.vector
    S = nc.scalar
    T = nc.tensor
    G = nc.gpsimd

    with TileContext(nc) as tc:
        with contextlib.ExitStack() as ctx:
            wp = ctx.enter_context(tc.tile_pool(name="w", bufs=1))
            xp = ctx.enter_context(tc.tile_pool(name="xp", bufs=1))
            plp = ctx.enter_context(tc.tile_pool(name="pl", bufs=2))
            wk = ctx.enter_context(tc.tile_pool(name="wk", bufs=2))
            psp = ctx.enter_context(tc.tile_pool(name="ps", bufs=2, space="PSUM"))

            # ---------------- one-time setup: weights -> diagonal matrices
            # identity matrix (bf16)
            ones = wp.tile([128, 128], dt, tag="ones")
            V.memset(ones[:], 1.0)
            ident = wp.tile([128, 128], dt, tag="ident")
            G.affine_select(
                ident[:], ones[:], [[1, 128]], OP.is_equal, 0.0,
                base=0, channel_multiplier=-1,
            )

            DG = []  # per cb: dict conv -> [128, 9, 128] diag tile
            BIAS = []  # per cb: dict conv -> [128, 1] f32 bias AP
            for cb in range(2):
                sl = slice(cb * 128, (cb + 1) * 128)
                wmg = wp.tile([128, 9], F32, tag=f"wmg{cb}")
                wpc = wp.tile([128, 36], F32, tag=f"wpc{cb}")
                wpg = wp.tile([128, 27], F32, tag=f"wpg{cb}")
                bmb = wp.tile([128, 1], F32, tag=f"bmb{cb}")
                bpb = wp.tile([128, 4], F32, tag=f"bpb{cb}")
                bgb = wp.tile([128, 3], F32, tag=f"bgb{cb}")
                nc.sync.dma_start(wmg[:], mg_d[sl, :])
                nc.sync.dma_start(wpc[:], pc_d[sl, :])
                nc.sync.dma_start(wpg[:], pg_d[sl, :])
                nc.sync.dma_start(bmb[:], mb_d[sl, :])
                nc.sync.dma_start(bpb[:], pb_d[sl, :])
                nc.sync.dma_start(bgb[:], gb_d[sl, :])

                # tap-weight difference vectors for the gating linearity trick
                wd01 = wp.tile([128, 9], F32, tag=f"wd01{cb}")
                wd23 = wp.tile([128, 9], F32, tag=f"wd23{cb}")
                V.tensor_tensor(wd01[:], wpc[:, 0:36:4], wpc[:, 1:36:4], OP.subtract)
                V.tensor_tensor(wd23[:], wpc[:, 2:36:4], wpc[:, 3:36:4], OP.subtract)
                bd01 = wp.tile([128, 1], F32, tag=f"bd01{cb}")
                bd23 = wp.tile([128, 1], F32, tag=f"bd23{cb}")
                V.tensor_tensor(bd01[:], bpb[:, 0:1], bpb[:, 1:2], OP.subtract)
                V.tensor_tensor(bd23[:], bpb[:, 2:3], bpb[:, 3:4], OP.subtract)

                # per-conv tap vector APs ([128, 9], may be strided views)
                wsrc = dict(
                    g0=wpg[:, 0:27:3],
                    d01=wd01[:],
                    p1=wpc[:, 1:36:4],
                    g1=wpg[:, 2:27:3],
                    d23=wd23[:],
                    p3=wpc[:, 3:36:4],
                    cm=wmg[:],
                )
                bias = dict(
                    g0=bgb[:, 0:1],
                    d01=bd01[:],
                    p1=bpb[:, 1:2],
                    g1=bgb[:, 1:2],
                    d23=bd23[:],
                    p3=bpb[:, 3:4],
                    cm=bmb[:, 0:1],
                    gc=bgb[:, 2:3],
                )
                dg = {}
                for cv in CONVS:
                    t = wp.tile([128, 9, 128], dt, tag=f"dg_{cb}_{cv}")
                    for tap in range(9):
                        V.tensor_scalar(
                            t[:, tap, :], ident[:], wsrc[cv][:, tap : tap + 1],
                            None, OP.mult,
                        )
                    dg[cv] = t
                DG.append(dg)
                BIAS.append(bias)

            # ---------------- per-plane pipeline
            def do_conv_s2(dgt, planes, sbuf_out, bias, func):
                """stride-2 3x3 depthwise conv via 18 PSUM half-tiles.

                dgt: [128, 9, 128] diagonal weights; planes: dict of parity
                plane tiles; sbuf_out: [128, 64, 64] bf16; bias: [128,1] f32.
                """
                ee, ezo, oe, ozo = planes["ee"], planes["ezo"], planes["oe"], planes["ozo"]
                for half in range(2):
                    ps = psp.tile([128, 32, 64], F32, tag="ps", name="ps")
                    r0 = half * 32
                    # taps ordered so (di=1, dj=1) comes first (full coverage,
                    # start=True clears each bank)
                    for ti, (di, dj) in enumerate(
                        [(1, 1), (1, 0), (1, 2), (0, 0), (0, 1), (0, 2),
                         (2, 0), (2, 1), (2, 2)]
                    ):
                        w = dgt[:, di * 3 + dj, :]
                        start = ti == 0
                        stop = ti == 8
                        for bk in range(4):
                            ob0 = bk * 8  # bank-local first out row
                            i0 = r0 + ob0  # global first out row of this bank
                            n_r = 8
                            o_off = 0
                            if di == 0 and i0 == 0:
                                # out row 0 has no di=0 contribution
                                n_r, o_off = 7, 1
                            # input rows in the parity plane
                            pr0 = i0 + o_off + (-1 if di == 0 else 0)
                            if di == 1:
                                src_t = ee if dj == 1 else ezo
                            else:
                                src_t = oe if dj == 1 else ozo
                            if dj == 1:
                                rhs = src_t[:, pr0 : pr0 + n_r, :]
                            else:
                                c0 = 0 if dj == 0 else 1
                                rhs = src_t[:, pr0 : pr0 + n_r, c0 : c0 + 64]
                            out = ps[:, ob0 + o_off : ob0 + o_off + n_r, :]
                            T.matmul(out, w, rhs, start=start, stop=stop,
                                     skip_group_check=True)
                    S.activation(
                        sbuf_out[:, r0 : r0 + 32, :], ps[:], func, bias=bias
                    )

            def do_conv_s1(dgt, n0z, sbuf_out, bias, func):
                """stride-1 3x3 depthwise conv over col-padded n0z [128,64,66]."""
                for half in range(2):
                    ps = psp.tile([128, 32, 64], F32, tag="ps", name="ps")
                    r0 = half * 32
                    for ti, (di, dj) in enumerate(
                        [(1, 1), (1, 0), (1, 2), (0, 0), (0, 1), (0, 2),
                         (2, 0), (2, 1), (2, 2)]
                    ):
                        w = dgt[:, di * 3 + dj, :]
                        start = ti == 0
                        stop = ti == 8
                        for bk in range(4):
                            ob0 = bk * 8
                            i0 = r0 + ob0
                            n_r = 8
                            o_off = 0
                            if di == 0 and i0 == 0:
                                n_r, o_off = 7, 1
                            if di == 2 and i0 + 8 == 64:
                                n_r = 7
                            pr0 = i0 + o_off + di - 1
                            rhs = n0z[:, pr0 : pr0 + n_r, dj : dj + 64]
                            out = ps[:, ob0 + o_off : ob0 + o_off + n_r, :]
                            T.matmul(out, w, rhs, start=start, stop=stop,
                                     skip_group_check=True)
                    S.activation(
                        sbuf_out[:, r0 : r0 + 32, :], ps[:], func, bias=bias
                    )

            def plane(b, cb):
                sl = slice(cb * 128, (cb + 1) * 128)
                dg = DG[cb]
                bias = BIAS[cb]

                X = xp.tile([128, H, H], dt, tag="X", name="X")
                G.dma_start(X[:], x_d[b, sl, :, :])  # casts f32->bf16

                # parity planes: ee[i,j]=x[2i,2j]; ezo[:, :, 1+j]=x[2i,2j+1],
                # col 0 = zeros (conv left-pad); oe/ozo likewise on odd rows.
                ee = plp.tile([128, 64, 64], dt, tag="ee", name="ee")
                ezo = plp.tile([128, 64, 65], dt, tag="ezo", name="ezo")
                oe = plp.tile([128, 64, 64], dt, tag="oe", name="oe")
                ozo = plp.tile([128, 64, 65], dt, tag="ozo", name="ozo")
                S.copy(ee[:], X[:, 0:128:2, 0:128:2])
                S.copy(ezo[:, :, 1:65], X[:, 0:128:2, 1:128:2])
                S.copy(oe[:], X[:, 1:128:2, 0:128:2])
                S.copy(ozo[:, :, 1:65], X[:, 1:128:2, 1:128:2])
                G.memset(ezo[:, :, 0:1], 0)
                G.memset(ozo[:, :, 0:1], 0)
                planes = dict(ee=ee, ezo=ezo, oe=oe, ozo=ozo)

                g0 = wk.tile([128, 64, 64], dt, tag="B", name="g0")
                do_conv_s2(dg["g0"], planes, g0, bias["g0"], AF.Sigmoid)
                d01 = wk.tile([128, 64, 64], dt, tag="C", name="d01")
                do_conv_s2(dg["d01"], planes, d01, bias["d01"], AF.Identity)
                p1 = wk.tile([128, 64, 64], dt, tag="D", name="p1")
                do_conv_s2(dg["p1"], planes, p1, bias["p1"], AF.Identity)

                # n0 = p1 + g0*d01 (zero-padded cols for the stride-1 conv)
                n0z = wk.tile([128, 64, 66], dt, tag="E", name="n0z", bufs=1)
                V.tensor_tensor(g0[:], g0[:], d01[:], OP.mult)
                G.memset(n0z[:, :, 0:1], 0)
                G.memset(n0z[:, :, 65:66], 0)
                n0 = n0z[:, :, 1:65]
                V.tensor_tensor(n0, p1[:], g0[:], OP.add)

                g1 = wk.tile([128, 64, 64], dt, tag="B", name="g1")
                do_conv_s2(dg["g1"], planes, g1, bias["g1"], AF.Sigmoid)
                d23 = wk.tile([128, 64, 64], dt, tag="C", name="d23")
                do_conv_s2(dg["d23"], planes, d23, bias["d23"], AF.Identity)
                p3 = wk.tile([128, 64, 64], dt, tag="D", name="p3")
                do_conv_s2(dg["p3"], planes, p3, bias["p3"], AF.Identity)

                # n1 = p3 + g1*d23  (into p3)
                V.tensor_tensor(g1[:], g1[:], d23[:], OP.mult)
                V.tensor_tensor(p3[:], p3[:], g1[:], OP.add)
                n1 = p3

                # node gate: stride-1 conv over n0 with g1's weights
                gc = wk.tile([128, 64, 64], dt, tag="B", name="gc")
                do_conv_s1(dg["g1"], n0z, gc, bias["gc"], AF.Sigmoid)

                # maxpool via tensor_tensor max chain on parity planes
                # (dj=0 taps must exclude the padded column: pad is -inf for max)
                m = wk.tile([128, 64, 64], dt, tag="F", name="m")
                V.tensor_tensor(m[:], ee[:], ezo[:, :, 1:65], OP.max)  # (1,1),(1,2)
                V.tensor_tensor(m[:, :, 1:64], m[:, :, 1:64], ezo[:, :, 1:64], OP.max)
                V.tensor_tensor(m[:], m[:], oe[:], OP.max)  # (2,1)
                V.tensor_tensor(m[:], m[:], ozo[:, :, 1:65], OP.max)  # (2,2)
                V.tensor_tensor(m[:, :, 1:64], m[:, :, 1:64], ozo[:, :, 1:64], OP.max)
                V.tensor_tensor(m[:, 1:64, :], m[:, 1:64, :], oe[:, 0:63, :], OP.max)
                V.tensor_tensor(
                    m[:, 1:64, :], m[:, 1:64, :], ozo[:, 0:63, 1:65], OP.max
                )
                V.tensor_tensor(
                    m[:, 1:64, 1:64], m[:, 1:64, 1:64], ozo[:, 0:63, 1:64], OP.max
                )

                # o = mpcm + n1 + gc*(n0 - n1)
                o = wk.tile([128, 64, 64], dt, tag="C", name="o")
                V.tensor_tensor(o[:], n0, n1[:], OP.subtract)
                V.tensor_tensor(o[:], o[:], gc[:], OP.mult)
                V.tensor_tensor(o[:], o[:], n1[:], OP.add)

                cm = wk.tile([128, 64, 64], dt, tag="B", name="cm")
                do_conv_s2(dg["cm"], planes, cm, bias["cm"], AF.Identity)
                V.tensor_tensor(m[:], m[:], cm[:], OP.mult)
                V.tensor_tensor(o[:], o[:], m[:], OP.add)

                oflat = o[:].rearrange("p a b -> p (a b)")
                G.dma_start(out_d[b, sl, :], oflat)  # cast bf16->f32

            rep_ctx = tc.For_i(0, reps, 1) if reps > 1 else contextlib.nullcontext()
            with rep_ctx:
                for b in range(BS):
                    for cb in range(2):
                        plane(b, cb)

    nc.compile()
    return nc


_NC_CACHE = {}


def _get_nc(dt=BF16, reps=1):
    key = (str(dt), reps)
    if key not in _NC_CACHE:
        _NC_CACHE[key] = _build(dt, reps)
    return _NC_CACHE[key]


def _in_maps(x, maxgate, mb, pconvs, pbs, pgates, gbs):
    x = np.ascontiguousarray(np.asarray(x, np.float32))
    maps = []
    for i in range(N_CORES):
        maps.append(
            dict(
                x=x[i * BS : (i + 1) * BS],
                maxgate=np.asarray(maxgate, np.float32).reshape(C, 9),
                mb=np.asarray(mb, np.float32).reshape(C, 1),
                pconvs=np.asarray(pconvs, np.float32).reshape(C, 36),
                pbs=np.asarray(pbs, np.float32).reshape(C, 4),
                pgates=np.asarray(pgates, np.float32).reshape(C, 27),
                gbs=np.asarray(gbs, np.float32).reshape(C, 3),
            )
        )
    return maps


def kernel(x, maxgate, mb, pconvs, pbs, pgates, gbs):
    nc = _get_nc(BF16)
    maps = _in_maps(x, maxgate, mb, pconvs, pbs, pgates, gbs)
    res = run_bass_kernel_spmd(nc, maps, list(range(N_CORES)))
    return np.concatenate(
        [r["out"].reshape(BS, C, HO, HO) for r in res.results], axis=0
    )


# revision 3
# speedup vs baseline: 1.7341x; 1.2112x over previous
"""Trainium2 Bass kernel v2 for nn_DenseNetCmaxGatedB2 (gated pooling block).

Computation (per batch, per channel, depthwise):
  out = maxpool3x3s2(x) * (dwconv_s2(x, maxgate) + mb)
  g0  = sigmoid(dwconv_s2(x, pgates[...,0]) + gbs[:,0])
  n0  = g0*p0 + (1-g0)*p1           p_k = dwconv_s2(x, pconvs[...,k]) + pbs[:,k]
  g1  = sigmoid(dwconv_s2(x, pgates[...,2]) + gbs[:,1])
  n1  = g1*p2 + (1-g1)*p3
  g   = sigmoid(dwconv_s1(n0, pgates[...,2]) + gbs[:,2])
  out = out + n0*g + n1*(1-g)

v2 strategy: all conv tap-MACs run on the TensorEngine as diagonal-weight
matmuls accumulating in fp32 PSUM (9 taps per conv; per-channel weight on
the diagonal of a 128x128 bf16 stationary matrix).  ScalarE evacuates
PSUM with fused bias (+sigmoid for gates) to bf16 SBUF.  DVE does the
maxpool (tensor_tensor max chain on parity planes) and gating combines.
Weight-linearity trick: n0 = p1 + g0*conv(x, w0-w1), so only 7 stride-2
convs + 1 stride-1 conv are needed.

Sharding: pure data parallel over batch (16 -> 2 per core, 8 cores).
"""

import contextlib
import sys

sys.path.insert(0, "/opt/trn_rl_repo")

import numpy as np

import concourse.bass as bass  # noqa: E402,F401
import concourse.mybir as mybir  # noqa: E402
from concourse import bacc  # noqa: E402
from concourse.tile import TileContext  # noqa: E402
from concourse.bass_utils import run_bass_kernel_spmd  # noqa: E402

N_CORES = 8
B, C, H = 16, 256, 128
HO = H // 2
BS = B // N_CORES  # batches per core
F32 = mybir.dt.float32
BF16 = mybir.dt.bfloat16
AF = mybir.ActivationFunctionType
OP = mybir.AluOpType

# conv definitions: name -> (weight source, bias source, sigmoid?)
# weight tap vectors are [128, 9] tiles built at setup per channel block.
CONVS = ["g0", "d01", "p1", "g1", "d23", "p3", "cm"]


def _build(dt=BF16, reps=1):
    """Build the SPMD program for one core (2 batches, full channels)."""
    nc = bacc.Bacc("TRN2", target_bir_lowering=False, debug=False, num_devices=N_CORES)

    x_d = nc.dram_tensor("x", [BS, C, H, H], F32, kind="ExternalInput")
    mg_d = nc.dram_tensor("maxgate", [C, 9], F32, kind="ExternalInput")
    mb_d = nc.dram_tensor("mb", [C, 1], F32, kind="ExternalInput")
    pc_d = nc.dram_tensor("pconvs", [C, 36], F32, kind="ExternalInput")
    pb_d = nc.dram_tensor("pbs", [C, 4], F32, kind="ExternalInput")
    pg_d = nc.dram_tensor("pgates", [C, 27], F32, kind="ExternalInput")
    gb_d = nc.dram_tensor("gbs", [C, 3], F32, kind="ExternalInput")
    out_d = nc.dram_tensor("out", [BS, C, HO * HO], F32, kind="ExternalOutput")

    V = nc.vector
    S = nc.scalar
    T = nc.tensor
    G = nc.gpsimd

    with TileContext(nc) as tc:
        with contextlib.ExitStack() as ctx:
            wp = ctx.enter_context(tc.tile_pool(name="w", bufs=1))
            xp = ctx.enter_context(tc.tile_pool(name="xp", bufs=1))
            plp = ctx.enter_context(tc.tile_pool(name="pl", bufs=2))
            wk = ctx.enter_context(tc.tile_pool(name="wk", bufs=2))
            psp = ctx.enter_context(tc.tile_pool(name="ps", bufs=2, space="PSUM"))

            # ---------------- one-time setup: weights -> diagonal matrices
            # identity matrix (bf16)
            ones = wp.tile([128, 128], dt, tag="ones")
            V.memset(ones[:], 1.0)
            ident = wp.tile([128, 128], dt, tag="ident")
            G.affine_select(
                ident[:], ones[:], [[1, 128]], OP.is_equal, 0.0,
                base=0, channel_multiplier=-1,
            )

            DG = []  # per cb: dict conv -> [128, 9, 128] diag tile
            BIAS = []  # per cb: dict conv -> [128, 1] f32 bias AP
            for cb in range(2):
                sl = slice(cb * 128, (cb + 1) * 128)
                wmg = wp.tile([128, 9], F32, tag=f"wmg{cb}")
                wpc = wp.tile([128, 36], F32, tag=f"wpc{cb}")
                wpg = wp.tile([128, 27], F32, tag=f"wpg{cb}")
                bmb = wp.tile([128, 1], F32, tag=f"bmb{cb}")
                bpb = wp.tile([128, 4], F32, tag=f"bpb{cb}")
                bgb = wp.tile([128, 3], F32, tag=f"bgb{cb}")
                nc.sync.dma_start(wmg[:], mg_d[sl, :])
                nc.sync.dma_start(wpc[:], pc_d[sl, :])
                nc.sync.dma_start(wpg[:], pg_d[sl, :])
                nc.sync.dma_start(bmb[:], mb_d[sl, :])
                nc.sync.dma_start(bpb[:], pb_d[sl, :])
                nc.sync.dma_start(bgb[:], gb_d[sl, :])

                # tap-weight difference vectors for the gating linearity trick
                wd01 = wp.tile([128, 9], F32, tag=f"wd01{cb}")
                wd23 = wp.tile([128, 9], F32, tag=f"wd23{cb}")
                V.tensor_tensor(wd01[:], wpc[:, 0:36:4], wpc[:, 1:36:4], OP.subtract)
                V.tensor_tensor(wd23[:], wpc[:, 2:36:4], wpc[:, 3:36:4], OP.subtract)
                bd01 = wp.tile([128, 1], F32, tag=f"bd01{cb}")
                bd23 = wp.tile([128, 1], F32, tag=f"bd23{cb}")
                V.tensor_tensor(bd01[:], bpb[:, 0:1], bpb[:, 1:2], OP.subtract)
                V.tensor_tensor(bd23[:], bpb[:, 2:3], bpb[:, 3:4], OP.subtract)

                # per-conv tap vector APs ([128, 9], may be strided views)
                wsrc = dict(
                    g0=wpg[:, 0:27:3],
                    d01=wd01[:],
                    p1=wpc[:, 1:36:4],
                    g1=wpg[:, 2:27:3],
                    d23=wd23[:],
                    p3=wpc[:, 3:36:4],
                    cm=wmg[:],
                )
                bias = dict(
                    g0=bgb[:, 0:1],
                    d01=bd01[:],
                    p1=bpb[:, 1:2],
                    g1=bgb[:, 1:2],
                    d23=bd23[:],
                    p3=bpb[:, 3:4],
                    cm=bmb[:, 0:1],
                    gc=bgb[:, 2:3],
                )
                dg = {}
                for cv in CONVS:
                    t = wp.tile([128, 9, 128], dt, tag=f"dg_{cb}_{cv}")
                    for tap in range(9):
                        V.tensor_scalar(
                            t[:, tap, :], ident[:], wsrc[cv][:, tap : tap + 1],
                            None, OP.mult,
                        )
                    dg[cv] = t
                DG.append(dg)
                BIAS.append(bias)

            # ---------------- per-plane pipeline
            def do_conv_s2(dgt, planes, sbuf_out, bias, func):
                """stride-2 3x3 depthwise conv via 18 PSUM half-tiles.

                dgt: [128, 9, 128] diagonal weights; planes: dict of parity
                plane tiles; sbuf_out: [128, 64, 64] bf16; bias: [128,1] f32.
                """
                ee, ezo, oe, ozo = planes["ee"], planes["ezo"], planes["oe"], planes["ozo"]
                for half in range(2):
                    ps = psp.tile([128, 32, 64], F32, tag="ps", name="ps")
                    r0 = half * 32
                    # taps ordered so (di=1, dj=1) comes first (full coverage,
                    # start=True clears each bank)
                    for ti, (di, dj) in enumerate(
                        [(1, 1), (1, 0), (1, 2), (0, 0), (0, 1), (0, 2),
                         (2, 0), (2, 1), (2, 2)]
                    ):
                        w = dgt[:, di * 3 + dj, :]
                        start = ti == 0
                        stop = ti == 8
                        for bk in range(4):
                            ob0 = bk * 8  # bank-local first out row
                            i0 = r0 + ob0  # global first out row of this bank
                            n_r = 8
                            o_off = 0
                            if di == 0 and i0 == 0:
                                # out row 0 has no di=0 contribution
                                n_r, o_off = 7, 1
                            # input rows in the parity plane
                            pr0 = i0 + o_off + (-1 if di == 0 else 0)
                            if di == 1:
                                src_t = ee if dj == 1 else ezo
                            else:
                                src_t = oe if dj == 1 else ozo
                            if dj == 1:
                                rhs = src_t[:, pr0 : pr0 + n_r, :]
                            else:
                                c0 = 0 if dj == 0 else 1
                                rhs = src_t[:, pr0 : pr0 + n_r, c0 : c0 + 64]
                            out = ps[:, ob0 + o_off : ob0 + o_off + n_r, :]
                            T.matmul(out, w, rhs, start=start, stop=stop,
                                     skip_group_check=True)
                    S.activation(
                        sbuf_out[:, r0 : r0 + 32, :], ps[:], func, bias=bias
                    )

            def do_conv_s1(dgt, n0z, sbuf_out, bias, func):
                """stride-1 3x3 depthwise conv over col-padded n0z [128,64,66]."""
                for half in range(2):
                    ps = psp.tile([128, 32, 64], F32, tag="ps", name="ps")
                    r0 = half * 32
                    for ti, (di, dj) in enumerate(
                        [(1, 1), (1, 0), (1, 2), (0, 0), (0, 1), (0, 2),
                         (2, 0), (2, 1), (2, 2)]
                    ):
                        w = dgt[:, di * 3 + dj, :]
                        start = ti == 0
                        stop = ti == 8
                        for bk in range(4):
                            ob0 = bk * 8
                            i0 = r0 + ob0
                            n_r = 8
                            o_off = 0
                            if di == 0 and i0 == 0:
                                n_r, o_off = 7, 1
                            if di == 2 and i0 + 8 == 64:
                                n_r = 7
                            pr0 = i0 + o_off + di - 1
                            rhs = n0z[:, pr0 : pr0 + n_r, dj : dj + 64]
                            out = ps[:, ob0 + o_off : ob0 + o_off + n_r, :]
                            T.matmul(out, w, rhs, start=start, stop=stop,
                                     skip_group_check=True)
                    S.activation(
                        sbuf_out[:, r0 : r0 + 32, :], ps[:], func, bias=bias
                    )

            def plane(b, cb):
                sl = slice(cb * 128, (cb + 1) * 128)
                dg = DG[cb]
                bias = BIAS[cb]

                X = xp.tile([128, H, H], dt, tag="X", name="X")
                G.dma_start(X[:], x_d[b, sl, :, :])  # casts f32->bf16

                # parity planes: ee[i,j]=x[2i,2j]; ezo[:, :, 1+j]=x[2i,2j+1],
                # col 0 = zeros (conv left-pad); oe/ozo likewise on odd rows.
                ee = plp.tile([128, 64, 64], dt, tag="ee", name="ee")
                ezo = plp.tile([128, 64, 65], dt, tag="ezo", name="ezo")
                oe = plp.tile([128, 64, 64], dt, tag="oe", name="oe")
                ozo = plp.tile([128, 64, 65], dt, tag="ozo", name="ozo")
                S.copy(ee[:], X[:, 0:128:2, 0:128:2])
                S.copy(ezo[:, :, 1:65], X[:, 0:128:2, 1:128:2])
                S.copy(oe[:], X[:, 1:128:2, 0:128:2])
                S.copy(ozo[:, :, 1:65], X[:, 1:128:2, 1:128:2])
                G.memset(ezo[:, :, 0:1], 0)
                G.memset(ozo[:, :, 0:1], 0)
                planes = dict(ee=ee, ezo=ezo, oe=oe, ozo=ozo)

                g0 = wk.tile([128, 64, 64], dt, tag="B", name="g0")
                do_conv_s2(dg["g0"], planes, g0, bias["g0"], AF.Sigmoid)
                d01 = wk.tile([128, 64, 64], dt, tag="C", name="d01")
                do_conv_s2(dg["d01"], planes, d01, bias["d01"], AF.Identity)
                p1 = wk.tile([128, 64, 64], dt, tag="D", name="p1")
                do_conv_s2(dg["p1"], planes, p1, bias["p1"], AF.Identity)

                # n0 = p1 + g0*d01 (zero-padded cols for the stride-1 conv)
                n0z = wk.tile([128, 64, 66], dt, tag="E", name="n0z", bufs=1)
                V.tensor_tensor(g0[:], g0[:], d01[:], OP.mult)
                G.memset(n0z[:, :, 0:1], 0)
                G.memset(n0z[:, :, 65:66], 0)
                n0 = n0z[:, :, 1:65]
                V.tensor_tensor(n0, p1[:], g0[:], OP.add)

                g1 = wk.tile([128, 64, 64], dt, tag="B", name="g1")
                do_conv_s2(dg["g1"], planes, g1, bias["g1"], AF.Sigmoid)
                d23 = wk.tile([128, 64, 64], dt, tag="C", name="d23")
                do_conv_s2(dg["d23"], planes, d23, bias["d23"], AF.Identity)
                p3 = wk.tile([128, 64, 64], dt, tag="D", name="p3")
                do_conv_s2(dg["p3"], planes, p3, bias["p3"], AF.Identity)

                # n1 = p3 + g1*d23  (into p3)
                V.tensor_tensor(g1[:], g1[:], d23[:], OP.mult)
                V.tensor_tensor(p3[:], p3[:], g1[:], OP.add)
                n1 = p3

                # node gate: stride-1 conv over n0 with g1's weights
                gc = wk.tile([128, 64, 64], dt, tag="B", name="gc")
                do_conv_s1(dg["g1"], n0z, gc, bias["gc"], AF.Sigmoid)

                # maxpool via tensor_tensor max chain on parity planes
                # (dj=0 taps must exclude the padded column: pad is -inf for max)
                m = wk.tile([128, 64, 64], dt, tag="F", name="m")
                V.tensor_tensor(m[:], ee[:], ezo[:, :, 1:65], OP.max)  # (1,1),(1,2)
                V.tensor_tensor(m[:, :, 1:64], m[:, :, 1:64], ezo[:, :, 1:64], OP.max)
                V.tensor_tensor(m[:], m[:], oe[:], OP.max)  # (2,1)
                V.tensor_tensor(m[:], m[:], ozo[:, :, 1:65], OP.max)  # (2,2)
                V.tensor_tensor(m[:, :, 1:64], m[:, :, 1:64], ozo[:, :, 1:64], OP.max)
                V.tensor_tensor(m[:, 1:64, :], m[:, 1:64, :], oe[:, 0:63, :], OP.max)
                V.tensor_tensor(
                    m[:, 1:64, :], m[:, 1:64, :], ozo[:, 0:63, 1:65], OP.max
                )
                V.tensor_tensor(
                    m[:, 1:64, 1:64], m[:, 1:64, 1:64], ozo[:, 0:63, 1:64], OP.max
                )

                # o = mpcm + n1 + gc*(n0 - n1)
                o = wk.tile([128, 64, 64], dt, tag="C", name="o")
                V.tensor_tensor(o[:], n0, n1[:], OP.subtract)
                V.tensor_tensor(o[:], o[:], gc[:], OP.mult)
                V.tensor_tensor(o[:], o[:], n1[:], OP.add)

                cm = wk.tile([128, 64, 64], dt, tag="B", name="cm")
                do_conv_s2(dg["cm"], planes, cm, bias["cm"], AF.Identity)
                V.tensor_tensor(m[:], m[:], cm[:], OP.mult)
                V.tensor_tensor(o[:], o[:], m[:], OP.add)

                oflat = o[:].rearrange("p a b -> p (a b)")
                G.dma_start(out_d[b, sl, :], oflat)  # cast bf16->f32

            rep_ctx = tc.For_i(0, reps, 1) if reps > 1 else contextlib.nullcontext()
            with rep_ctx:
                for b in range(BS):
                    for cb in range(2):
                        plane(b, cb)

    _dedup_ldweights(nc)
    nc.compile()
    return nc


def _ldw_sig(ins):
    """Signature of an Ldweights instruction: weights AP + mode flags."""
    try:
        ap = ins.ins[0]
        return (
            str(ap),
            str(ins.perf_mode),
            str(ins.is_transpose),
            str(ins.tile_position),
            str(ins.tile_size),
        )
    except Exception:
        return None


def _dedup_ldweights(nc):
    """Remove Ldweights whose weights match the previous load on the PE
    queue (the tile lowering emits one per matmul unconditionally; 4
    consecutive matmuls here share the same diagonal weights).  Waits on a
    removed Ldweights migrate to the next kept instruction on the PE queue.
    Must run before nc.compile() (whose passes move matmul waits onto
    ldweights)."""
    n_removed = 0
    for fn in nc.m.functions:
        for blk in fn.blocks:
            insts = blk.instructions  # live list proxy
            last_sig = None
            last_keeper = None
            victims = []
            pending_waits = []
            remap = {}
            for ins in list(insts):
                op = ins.opcode
                si = ins.sync_info
                if op == "Ldweights":
                    sig = _ldw_sig(ins)
                    has_upd = si is not None and len(si.on_update) > 0
                    if sig is not None and sig == last_sig and not has_upd:
                        victims.append(ins)
                        remap[ins.name] = last_keeper
                        if si is not None and len(si.on_wait) > 0:
                            pending_waits.extend(si.on_wait)
                        continue
                    last_sig = sig
                    last_keeper = ins.name
                elif op == "Matmult":
                    if pending_waits:
                        if si is None:
                            ins.sync_info = mybir.SyncInfo(
                                on_wait=list(pending_waits), on_update=[]
                            )
                        else:
                            si.on_wait = list(si.on_wait) + pending_waits
                        pending_waits = []
                elif str(getattr(ins, "engine", "")) == "EngineType.PE":
                    last_sig = None
            assert not pending_waits, "dangling waits from removed Ldweights"
            for v in victims:
                insts.remove(v)
                n_removed += 1
            if remap:
                for ins in insts:
                    try:
                        ins.remap_dependency_names(remap)
                    except Exception:
                        pass
    return n_removed


_NC_CACHE = {}


def _get_nc(dt=BF16, reps=1):
    key = (str(dt), reps)
    if key not in _NC_CACHE:
        _NC_CACHE[key] = _build(dt, reps)
    return _NC_CACHE[key]


def _in_maps(x, maxgate, mb, pconvs, pbs, pgates, gbs):
    x = np.ascontiguousarray(np.asarray(x, np.float32))
    maps = []
    for i in range(N_CORES):
        maps.append(
            dict(
                x=x[i * BS : (i + 1) * BS],
                maxgate=np.asarray(maxgate, np.float32).reshape(C, 9),
                mb=np.asarray(mb, np.float32).reshape(C, 1),
                pconvs=np.asarray(pconvs, np.float32).reshape(C, 36),
                pbs=np.asarray(pbs, np.float32).reshape(C, 4),
                pgates=np.asarray(pgates, np.float32).reshape(C, 27),
                gbs=np.asarray(gbs, np.float32).reshape(C, 3),
            )
        )
    return maps


def kernel(x, maxgate, mb, pconvs, pbs, pgates, gbs):
    nc = _get_nc(BF16)
    maps = _in_maps(x, maxgate, mb, pconvs, pbs, pgates, gbs)
    res = run_bass_kernel_spmd(nc, maps, list(range(N_CORES)))
    return np.concatenate(
        [r["out"].reshape(BS, C, HO, HO) for r in res.results], axis=0
    )
